# revision 76
# baseline (speedup 1.0000x reference)
"""Pair-symmetry loss kernel, 244.7us (TimelineSim) vs 282.7us baseline.

Math: w(p,s)*|dsal(p,s)| is symmetric under (p,s)->(p+s,-s), so each of the
60 shift pairs is computed once on an extended domain (rows -6..353, cols
-5..356) and accumulated twice into PSUM via the TensorEngine.

Key differences from the v2 baseline:
  - per-channel Gaussian in ONE activation pass via
    Derivative_Erf(sqrt(2a)*d) = (2/sqrt(pi))*exp(-a*d^2), replacing
    Square + channel-sum adds + Exp (saves ~60us of ACT); channels combine
    with two tensor muls and (sqrt(pi)/2)^3 folds into the host division.
  - mirror accumulation per pair with superdiagonal shifted-identity
    matmuls (psA[p,j] += P[p+dp,jp]) instead of psG -> SBUF -> row-shift
    DMA -> psA group merges.
  - cho (odd-parity copy, rows 5..12 only) via SBUF->SBUF DMA off ACT;
    only rows actually read are loaded (rgb 5..12, sal 3..12).
  - per-pair column trim to the needed union [lo, lo+352+|sx|) of the
    direct and mirror ranges (avg width 355.5 vs 362, ~1.8% off every
    per-pair op on all three engines); pair 0 runs full width so its
    start=True matmuls reset all PSUM columns.
  - engine split at the measured optimum: abs 2/3 ACT / 1/3 DVE, first
    Gaussian mul alternates Pool/DVE 30/30, dsal on Pool, P-mul on DVE;
    mask ladders emitted staggered mid-pair-loop (pi=24/42/56; still descending at step +4 -- try later points first with more budget) as gap filler; final reduce fuses
    the mask multiply into the PSUM evacuation.

Layout: 120 partitions x 3 payload rows (global row 3p-6+j), per-channel
local window fp16, cols idx t <-> global col t-10.

Engine busy (sim): DVE ~220us, ACT ~218us, Pool ~202us, PE ~100us. The
split is a sharp optimum: shifting chain ops (P-mul, or >30 of the first
Gaussian muls) to Pool inserts bubbles into the serial PSUM accumulation
chain and costs ~10us despite lower engine-busy totals. Pitfalls the
simulator accepts but hardware rejects: multi-bank PSUM matmul outputs,
Pool TensorScalarPtr, gpsimd XYZWC reduce, memsets at partition>0,
gpsimd accumulate-DMA.
"""

import numpy as np

H = W = 352
RADIUS = 5
NP = 120                 # partitions; payload rows 3p-6 .. 3p-4
PADW2 = W + 20           # 372 : cols idx t <-> global col t-10
LROWS = 13               # local rows k <-> global row 3p-11+k
CH = LROWS * PADW2       # 4836 elements per channel
PW = W + 2 * RADIUS      # 362 : P/ssq domain, col q <-> global col q-5
N_CORES = 8

_CACHE = {}


def _build_kernel():
    from contextlib import ExitStack

    import concourse.bass as bass
    import concourse.tile as tile
    from concourse import bacc, mybir

    f16 = mybir.dt.float16
    f32 = mybir.dt.float32
    i16 = mybir.dt.int16
    Alu = mybir.AluOpType
    Act = mybir.ActivationFunctionType

    nc = bacc.Bacc(
        "TRN2",
        debug=False,
        enable_asserts=False,
        target_bir_lowering=False,
        num_devices=1,
        enable_partition_id=False,
    )
    # host-padded fp16 inputs: row r <-> global row r-11, col t <-> global t-10
    pred_d = nc.dram_tensor("pred16", [370, PADW2], f16, kind="ExternalInput")
    feat_d = nc.dram_tensor("feat16", [3, 370, PADW2], f16, kind="ExternalInput")
    out_d = nc.dram_tensor("partial", [NP, 2], f32, kind="ExternalOutput")

    with tile.TileContext(nc) as tc, ExitStack() as ctx:
        persist = ctx.enter_context(tc.tile_pool(name="persist", bufs=1))

        # all 4 channels in one tile; odd-shifted copy of the rgb channels.
        # Only the rows actually read are loaded: rgb rows 5..12, sal 3..12.
        ch4 = persist.tile([NP, 4, LROWS, PADW2], f16, tag="ch4")
        cho = persist.tile([NP, 3, LROWS - 5, PADW2], f16, tag="cho")

        # sal (c=3) first so the mask pipeline overlaps the rgb loads
        for c, k0 in ((3, 3), (0, 5), (1, 5), (2, 5)):
            src_ap = pred_d.ap() if c == 3 else feat_d.ap()[c]
            src = bass.AP(
                tensor=src_ap.tensor,
                offset=src_ap.offset + k0 * PADW2,
                ap=[[3 * PADW2, NP], [PADW2, LROWS - k0], [1, PADW2]],
            )
            nc.sync.dma_start(out=ch4[:, c, k0:LROWS, :], in_=src)
        ch4f = ch4[:].rearrange("p c a b -> p (c a b)")
        chof = cho[:].rearrange("p c a b -> p (c a b)")
        # odd-parity copy via SBUF->SBUF DMA (off the compute engines);
        # cho row k holds ch4 row k+5 shifted left one element
        nrow = (LROWS - 5) * PADW2
        for c in range(3):
            base = c * CH + 5 * PADW2
            nc.sync.dma_start(
                out=chof[:, c * nrow : (c + 1) * nrow],
                in_=ch4f[:, base + 1 : base + nrow + 1],
            )

        # ---- contour mask (in P-column coords, [NP, 3, 362]) ----
        # sal rows 0..2 are not loaded; threshold only rows 3..12
        zeros = persist.tile([1, 5 * PADW2], f16, tag="zeros")
        nc.gpsimd.memset(zeros[:], 0.0)

        # thresholds early; rows 3..10 (local 0..7) are all the ladder reads
        MR = 8
        salr = ch4[:, 3, 3 : 3 + MR, :].rearrange("p a b -> p (a b)")
        mp = ctx.enter_context(tc.tile_pool(name="maskpool", bufs=1))
        lbl = mp.tile([NP, MR, PADW2], f16, tag="lbl")
        nc.vector.tensor_scalar(
            out=lbl[:].rearrange("p a b -> p (a b)"), in0=salr,
            scalar1=0.5, scalar2=None, op0=Alu.is_gt,
        )
        u = mp.tile([NP, MR, PADW2], f16, tag="u")
        nc.vector.tensor_scalar(
            out=u[:].rearrange("p a b -> p (a b)"), in0=salr,
            scalar1=0.5, scalar2=None, op0=Alu.is_le,
        )
        # invalidate u outside the image: pad cols, then the boundary rows
        # the +-2 pool windows can reach (memset can't start at partition>0,
        # so boundary partitions use DMA from the zeros tile)
        nc.gpsimd.memset(u[:, :, 0:10], 0.0)
        nc.gpsimd.memset(u[:, :, 362:372], 0.0)
        nc.gpsimd.memset(u[0:1, :, :], 0.0)
        nc.sync.dma_start(out=u[1:2, 0:5, :], in_=zeros[:, 0 : 5 * PADW2])
        nc.sync.dma_start(out=u[2:3, 0:2, :], in_=zeros[:, 0 : 2 * PADW2])
        nc.sync.dma_start(out=u[118:119, 6:8, :], in_=zeros[:, 0 : 2 * PADW2])
        nc.sync.dma_start(out=u[119:120, 3:8, :], in_=zeros[:, 0 : 5 * PADW2])

        sums = persist.tile([NP, 2], f32, tag="sums")
        nc.gpsimd.memset(sums[:], 0.0)
        mask = persist.tile([NP, 3, PW], f16, tag="mask")

        ladA = mp.tile([NP, 6, PADW2], f16, tag="ladA")
        ladB = mp.tile([NP, 3, PADW2], f16, tag="ladB")
        rowm = mp.tile([NP, 3, PADW2], f16, tag="rowm")
        ladC = mp.tile([NP, 3, PW + 2], f16, tag="ladC")
        ladD = mp.tile([NP, 3, PW], f16, tag="ladD")
        dil = mp.tile([NP, 3, PW], f16, tag="dil")
        umax = mp.tile([NP, 3, PW], f16, tag="umax")

        def emit_ladder(srcb, dstb):
            """One 5x5-maxpool ladder; emitted mid-pair-loop so the DVE work
            lands in gaps instead of one burst."""
            eng = nc.vector
            eng.tensor_max(ladA[:], srcb[:, 0:6, :], srcb[:, 1:7, :])
            eng.tensor_max(ladB[:], ladA[:, 0:3, :], ladA[:, 2:5, :])
            eng.tensor_max(rowm[:], ladB[:], srcb[:, 4:7, :])
            # cols: dil[q] = max rowm[t=q+3..q+7]
            eng.tensor_max(ladC[:], rowm[:, :, 3 : 3 + PW + 2],
                           rowm[:, :, 4 : 4 + PW + 2])
            eng.tensor_max(ladD[:], ladC[:, :, 0:PW], ladC[:, :, 2 : 2 + PW])
            eng.tensor_max(dstb[:], ladD[:], rowm[:, :, 7 : 7 + PW])

        def emit_mask_finish():
            nc.vector.scalar_tensor_tensor(
                out=mask[:].rearrange("p a b -> p (a b)"),
                in0=dil[:].rearrange("p a b -> p (a b)"),
                scalar=-1.0,
                in1=umax[:].rearrange("p a b -> p (a b)"),
                op0=Alu.add, op1=Alu.add,
            )
            # zero mask outside the image: pad cols, partitions, junk rows
            nc.gpsimd.memset(mask[:, :, 0:RADIUS], 0.0)
            nc.gpsimd.memset(mask[:, :, RADIUS + W : PW], 0.0)
            nc.gpsimd.memset(mask[0:2, :, :], 0.0)
            nc.sync.dma_start(out=mask[119:120, 1:3, :], in_=zeros[:, 0 : 2 * PW])
            # mask sum per partition (placed mid-loop, fills a DVE gap)
            nc.vector.tensor_reduce(
                out=sums[:, 1:2], in_=mask[:].rearrange("p a b -> p (a b)"),
                axis=mybir.AxisListType.X, op=Alu.add,
            )

        # ---- identity + shifted identities for PE accumulation ----
        # ident_shift[a][k, m] = 1 iff m == k + a  (out[m] += P[m - a])
        rowidx = persist.tile([NP, NP], i16, tag="rowidx")
        pidx = persist.tile([NP, 1], mybir.dt.int32, tag="pidx")
        pidxf = persist.tile([NP, 1], f32, tag="pidxf")
        nc.gpsimd.iota(rowidx[:], pattern=[[1, NP]], base=0, channel_multiplier=0)
        nc.gpsimd.iota(pidx[:], pattern=[[1, 1]], base=0, channel_multiplier=1)
        nc.vector.tensor_copy(out=pidxf[:], in_=pidx[:])
        ident_shift = []
        for a in range(3):
            ida = persist.tile([NP, NP], f16, tag=f"ident{a}", name=f"ident{a}")
            pa = pidxf
            if a > 0:
                pa = persist.tile([NP, 1], f32, tag=f"pidxf{a}", name=f"pidxf{a}")
                nc.vector.tensor_scalar(
                    out=pa[:], in0=pidxf[:], scalar1=float(a), scalar2=None,
                    op0=Alu.add,
                )
            nc.vector.tensor_scalar(
                out=ida[:], in0=rowidx[:], scalar1=pa[:], scalar2=None,
                op0=Alu.is_equal,
            )
            ident_shift.append(ida)
        ident = ident_shift[0]

        pp = ctx.enter_context(tc.tile_pool(name="ps", bufs=1, space="PSUM"))
        psA = pp.tile([NP, 3, 512], f32, tag="psA")

        tmp = ctx.enter_context(tc.tile_pool(name="tmp", bufs=3))

        # half set: sy>0 all sx; sy=0 positive sx
        pairs = [(sy, sx) for sy in range(5, 0, -1) for sx in range(-5, 6)]
        pairs += [(0, sx) for sx in range(1, 6)]

        n_mm = 0

        def acc(out, lhsT, rhs, last):
            nonlocal n_mm
            # first pair's direct j=0,1,2 reset the three psA banks
            nc.tensor.matmul(
                out=out, lhsT=lhsT, rhs=rhs,
                start=(n_mm < 3), stop=last,
                skip_group_check=True,
            )
            n_mm += 1

        import math

        SQRT_ALPHA = math.sqrt(200.0)

        for pi, (sy, sx) in enumerate(pairs):
            if pi == 28:
                emit_ladder(lbl, dil)
            elif pi == 46:
                emit_ladder(u, umax)
            elif pi == 58:
                emit_mask_finish()
            # Column trim: pair columns q (global col q-5) are only needed on
            # the union of the mask-alive direct range [5,357) and the mirror
            # source range [5-sx, 357-sx); compute [lo, lo+wid) with lo even
            # (keeps the window views parity-aligned). Pair 0 runs full width
            # so its start=True matmuls reset every psA column.
            if pi == 0:
                lo, wid = 0, PW
            elif sx > 0:
                lo = (5 - sx) & ~1
                wid = 357 - lo
            else:
                lo = 4
                wid = 353 - sx

            off_par = (5 + sx) % 2
            if off_par == 0:
                c0 = 5 + sx + lo
                winr = ch4[:, 0:3, 5 + sy : 8 + sy, c0 : c0 + wid]
            else:
                c0 = 4 + sx + lo
                winr = cho[:, 0:3, sy : 3 + sy, c0 : c0 + wid]
            ctr = cho[:, 0:3, 0:3, 4 + lo : 4 + lo + wid]

            d4 = tmp.tile([NP, 3, 3, PW], f16, tag="d4")
            nc.vector.tensor_sub(d4[:, :, :, 0:wid], winr, ctr)
            # per-channel Gaussian in one ACT pass:
            # DerErf(sqrt(200)*d) = (2/sqrt(pi)) * exp(-200*d^2)
            # (the constant (sqrt(pi)/2)^3 is folded in on the host)
            G = tmp.tile([NP, 3, 3, PW], f16, tag="G")
            nc.scalar.activation(out=G[:, :, :, 0:wid], in_=d4[:, :, :, 0:wid],
                                 func=Act.Derivative_Erf, scale=SQRT_ALPHA)
            Ga = tmp.tile([NP, 3, PW], f16, tag="Ga")
            eng_m1 = nc.gpsimd if pi % 2 == 0 else nc.vector
            eng_m1.tensor_mul(Ga[:, :, 0:wid], G[:, 0, :, 0:wid],
                              G[:, 1, :, 0:wid])
            wgt = tmp.tile([NP, 3, PW], f16, tag="wgt")
            nc.vector.tensor_mul(wgt[:, :, 0:wid], Ga[:, :, 0:wid],
                                 G[:, 2, :, 0:wid])

            dsal = tmp.tile([NP, 3, PW], f16, tag="dsal")
            nc.gpsimd.tensor_sub(
                dsal[:, :, 0:wid],
                ch4[:, 3, 5 + sy : 8 + sy, 5 + sx + lo : 5 + sx + lo + wid],
                ch4[:, 3, 5:8, 5 + lo : 5 + lo + wid],
            )
            adsal = tmp.tile([NP, 3, PW], f16, tag="adsal")
            if pi % 3 == 2:
                nc.vector.tensor_scalar(
                    out=adsal[:, :, 0:wid].bitcast(mybir.dt.uint16),
                    in0=dsal[:, :, 0:wid].bitcast(mybir.dt.uint16),
                    scalar1=0x7FFF, scalar2=None, op0=Alu.bitwise_and,
                )
            else:
                nc.scalar.activation(out=adsal[:, :, 0:wid],
                                     in_=dsal[:, :, 0:wid], func=Act.Abs)
            P = tmp.tile([NP, 3, PW], f16, tag="P")
            nc.vector.tensor_mul(P[:, :, 0:wid], wgt[:, :, 0:wid],
                                 adsal[:, :, 0:wid])

            # PE accumulation (one matmul per psA bank; a matmul's PSUM
            # output may not span banks). Direct, then mirror: row-shift by
            # sy via shifted-identity lhsT (psA[p,j] += P[p+dp, jp],
            # jp=(j-sy)%3, dp=(j-sy-jp)/3); col-shift by -sx via rhs view.
            mms = [(0, psA[:, j, lo : lo + wid], ident[:], P[:, j, 0:wid])
                   for j in range(3)]
            mlo = RADIUS - sx - lo  # mirror source col within the trimmed P
            for j in range(3):
                jp = (j - sy) % 3
                dp = (j - sy - jp) // 3
                a = -dp
                mms.append((a, psA[:, j, RADIUS : RADIUS + W],
                            ident_shift[a][:],
                            P[:, jp, mlo : mlo + W]))
            mms.sort(key=lambda t: t[0])
            is_last_pair = (sy, sx) == pairs[-1]
            for mi, (_, out_ap, lhsT, rhs) in enumerate(mms):
                acc(out_ap, lhsT, rhs, is_last_pair and mi == len(mms) - 1)

        # ---- masked partial sum: evacuate PSUM with the mask multiply
        # fused into the same tensor_tensor, then reduce ----
        scratch = persist.tile([NP, 3, PW], f16, tag="scratch")
        nc.vector.tensor_mul(scratch[:], psA[:, 0:3, 0:PW], mask[:])
        nc.vector.tensor_reduce(
            out=sums[:, 0:1], in_=scratch[:].rearrange("p a b -> p (a b)"),
            axis=mybir.AxisListType.X, op=Alu.add,
        )
        nc.sync.dma_start(out=out_d.ap(), in_=sums[:])

    nc.compile()
    return nc


def kernel(pred, feat):
    import os

    # A stale PJRT compilation-cache hit was observed to return a bad
    # executable (NaN result); force a fresh compile per process.
    os.environ.setdefault("JAX_ENABLE_COMPILATION_CACHE", "false")
    try:
        import jax

        jax.config.update("jax_enable_compilation_cache", False)
    except Exception:
        pass

    if "nc" not in _CACHE:
        _CACHE["nc"] = _build_kernel()
    nc = _CACHE["nc"]
    from concourse.bass_utils import run_bass_kernel_spmd

    pred = np.asarray(pred, dtype=np.float32).reshape(N_CORES, H, W)
    feat = np.asarray(feat, dtype=np.float32).reshape(N_CORES, 3, H, W)
    predp = np.zeros((N_CORES, 370, PADW2), np.float16)
    predp[:, 11:363, 10:362] = pred.astype(np.float16)
    featp = np.zeros((N_CORES, 3, 370, PADW2), np.float16)
    featp[:, :, 11:363, 10:362] = feat.astype(np.float16)
    in_maps = [
        {"pred16": np.ascontiguousarray(predp[i]),
         "feat16": np.ascontiguousarray(featp[i])}
        for i in range(N_CORES)
    ]
    res = run_bass_kernel_spmd(nc, in_maps, core_ids=list(range(N_CORES)))
    _CACHE["last_results"] = res
    tot = np.zeros(2, np.float64)
    for r in res.results:
        tot += r["partial"].astype(np.float64).sum(axis=0)
    # kernel accumulates (2/sqrt(pi))^3 * exp(-alpha*ssq) via Derivative_Erf
    import math

    tot[0] *= (math.sqrt(math.pi) / 2.0) ** 3
    loss = tot[0] / (tot[1] + 1e-6)
    return np.array(loss, dtype=np.float32)


# revision 78
# speedup vs baseline: 1.0034x; 1.0034x over previous
"""Pair-symmetry loss kernel, 244.7us (TimelineSim) vs 282.7us baseline.

Math: w(p,s)*|dsal(p,s)| is symmetric under (p,s)->(p+s,-s), so each of the
60 shift pairs is computed once on an extended domain (rows -6..353, cols
-5..356) and accumulated twice into PSUM via the TensorEngine.

Key differences from the v2 baseline:
  - per-channel Gaussian in ONE activation pass via
    Derivative_Erf(sqrt(2a)*d) = (2/sqrt(pi))*exp(-a*d^2), replacing
    Square + channel-sum adds + Exp (saves ~60us of ACT); channels combine
    with two tensor muls and (sqrt(pi)/2)^3 folds into the host division.
  - mirror accumulation per pair with superdiagonal shifted-identity
    matmuls (psA[p,j] += P[p+dp,jp]) instead of psG -> SBUF -> row-shift
    DMA -> psA group merges.
  - cho (odd-parity copy, rows 5..12 only) via SBUF->SBUF DMA off ACT;
    only rows actually read are loaded (rgb 5..12, sal 3..12).
  - per-pair column trim to the needed union [lo, lo+352+|sx|) of the
    direct and mirror ranges (avg width 355.5 vs 362, ~1.8% off every
    per-pair op on all three engines); pair 0 runs full width so its
    start=True matmuls reset all PSUM columns.
  - engine split at the measured optimum: abs 2/3 ACT / 1/3 DVE, first
    Gaussian mul alternates Pool/DVE 30/30, dsal on Pool, P-mul on DVE;
    mask ladders emitted staggered mid-pair-loop (pi=24/42/56; still descending at step +4 -- try later points first with more budget) as gap filler; final reduce fuses
    the mask multiply into the PSUM evacuation.

Layout: 120 partitions x 3 payload rows (global row 3p-6+j), per-channel
local window fp16, cols idx t <-> global col t-10.

Engine busy (sim): DVE ~220us, ACT ~218us, Pool ~202us, PE ~100us. The
split is a sharp optimum: shifting chain ops (P-mul, or >30 of the first
Gaussian muls) to Pool inserts bubbles into the serial PSUM accumulation
chain and costs ~10us despite lower engine-busy totals. Pitfalls the
simulator accepts but hardware rejects: multi-bank PSUM matmul outputs,
Pool TensorScalarPtr, gpsimd XYZWC reduce, memsets at partition>0,
gpsimd accumulate-DMA.
"""

import numpy as np

H = W = 352
RADIUS = 5
NP = 120                 # partitions; payload rows 3p-6 .. 3p-4
PADW2 = W + 20           # 372 : cols idx t <-> global col t-10
LROWS = 13               # local rows k <-> global row 3p-11+k
CH = LROWS * PADW2       # 4836 elements per channel
PW = W + 2 * RADIUS      # 362 : P/ssq domain, col q <-> global col q-5
N_CORES = 8

_CACHE = {}


def _build_kernel():
    from contextlib import ExitStack

    import concourse.bass as bass
    import concourse.tile as tile
    from concourse import bacc, mybir

    f16 = mybir.dt.float16
    f32 = mybir.dt.float32
    i16 = mybir.dt.int16
    Alu = mybir.AluOpType
    Act = mybir.ActivationFunctionType

    nc = bacc.Bacc(
        "TRN2",
        debug=False,
        enable_asserts=False,
        target_bir_lowering=False,
        num_devices=1,
        enable_partition_id=False,
    )
    # host-padded fp16 inputs: row r <-> global row r-11, col t <-> global t-10
    pred_d = nc.dram_tensor("pred16", [370, PADW2], f16, kind="ExternalInput")
    feat_d = nc.dram_tensor("feat16", [3, 370, PADW2], f16, kind="ExternalInput")
    out_d = nc.dram_tensor("partial", [NP, 2], f32, kind="ExternalOutput")

    with tile.TileContext(nc) as tc, ExitStack() as ctx:
        persist = ctx.enter_context(tc.tile_pool(name="persist", bufs=1))

        # all 4 channels in one tile; odd-shifted copy of the rgb channels.
        # Only the rows actually read are loaded: rgb rows 5..12, sal 3..12.
        ch4 = persist.tile([NP, 4, LROWS, PADW2], f16, tag="ch4")
        cho = persist.tile([NP, 3, LROWS - 5, PADW2], f16, tag="cho")

        # sal (c=3) first so the mask pipeline overlaps the rgb loads
        for c, k0 in ((3, 3), (0, 5), (1, 5), (2, 5)):
            src_ap = pred_d.ap() if c == 3 else feat_d.ap()[c]
            src = bass.AP(
                tensor=src_ap.tensor,
                offset=src_ap.offset + k0 * PADW2,
                ap=[[3 * PADW2, NP], [PADW2, LROWS - k0], [1, PADW2]],
            )
            nc.sync.dma_start(out=ch4[:, c, k0:LROWS, :], in_=src)
        ch4f = ch4[:].rearrange("p c a b -> p (c a b)")
        chof = cho[:].rearrange("p c a b -> p (c a b)")
        # odd-parity copy via SBUF->SBUF DMA (off the compute engines);
        # cho row k holds ch4 row k+5 shifted left one element
        nrow = (LROWS - 5) * PADW2
        for c in range(3):
            base = c * CH + 5 * PADW2
            nc.sync.dma_start(
                out=chof[:, c * nrow : (c + 1) * nrow],
                in_=ch4f[:, base + 1 : base + nrow + 1],
            )

        # ---- contour mask (in P-column coords, [NP, 3, 362]) ----
        # sal rows 0..2 are not loaded; threshold only rows 3..12
        zeros = persist.tile([1, 5 * PADW2], f16, tag="zeros")
        nc.gpsimd.memset(zeros[:], 0.0)

        # thresholds early; rows 3..10 (local 0..7) are all the ladder reads
        MR = 8
        salr = ch4[:, 3, 3 : 3 + MR, :].rearrange("p a b -> p (a b)")
        mp = ctx.enter_context(tc.tile_pool(name="maskpool", bufs=1))
        lbl = mp.tile([NP, MR, PADW2], f16, tag="lbl")
        nc.vector.tensor_scalar(
            out=lbl[:].rearrange("p a b -> p (a b)"), in0=salr,
            scalar1=0.5, scalar2=None, op0=Alu.is_gt,
        )
        u = mp.tile([NP, MR, PADW2], f16, tag="u")
        nc.vector.tensor_scalar(
            out=u[:].rearrange("p a b -> p (a b)"), in0=salr,
            scalar1=0.5, scalar2=None, op0=Alu.is_le,
        )
        # invalidate u outside the image: pad cols, then the boundary rows
        # the +-2 pool windows can reach (memset can't start at partition>0,
        # so boundary partitions use DMA from the zeros tile)
        nc.gpsimd.memset(u[:, :, 0:10], 0.0)
        nc.gpsimd.memset(u[:, :, 362:372], 0.0)
        nc.gpsimd.memset(u[0:1, :, :], 0.0)
        nc.sync.dma_start(out=u[1:2, 0:5, :], in_=zeros[:, 0 : 5 * PADW2])
        nc.sync.dma_start(out=u[2:3, 0:2, :], in_=zeros[:, 0 : 2 * PADW2])
        nc.sync.dma_start(out=u[118:119, 6:8, :], in_=zeros[:, 0 : 2 * PADW2])
        nc.sync.dma_start(out=u[119:120, 3:8, :], in_=zeros[:, 0 : 5 * PADW2])

        sums = persist.tile([NP, 2], f32, tag="sums")
        nc.gpsimd.memset(sums[:], 0.0)
        mask = persist.tile([NP, 3, PW], f16, tag="mask")

        ladA = mp.tile([NP, 6, PADW2], f16, tag="ladA")
        ladB = mp.tile([NP, 3, PADW2], f16, tag="ladB")
        rowm = mp.tile([NP, 3, PADW2], f16, tag="rowm")
        ladC = mp.tile([NP, 3, PW + 2], f16, tag="ladC")
        ladD = mp.tile([NP, 3, PW], f16, tag="ladD")
        dil = mp.tile([NP, 3, PW], f16, tag="dil")
        umax = mp.tile([NP, 3, PW], f16, tag="umax")

        def emit_ladder(srcb, dstb):
            """One 5x5-maxpool ladder; emitted mid-pair-loop so the DVE work
            lands in gaps instead of one burst."""
            eng = nc.vector
            eng.tensor_max(ladA[:], srcb[:, 0:6, :], srcb[:, 1:7, :])
            eng.tensor_max(ladB[:], ladA[:, 0:3, :], ladA[:, 2:5, :])
            eng.tensor_max(rowm[:], ladB[:], srcb[:, 4:7, :])
            # cols: dil[q] = max rowm[t=q+3..q+7]
            eng.tensor_max(ladC[:], rowm[:, :, 3 : 3 + PW + 2],
                           rowm[:, :, 4 : 4 + PW + 2])
            eng.tensor_max(ladD[:], ladC[:, :, 0:PW], ladC[:, :, 2 : 2 + PW])
            eng.tensor_max(dstb[:], ladD[:], rowm[:, :, 7 : 7 + PW])

        def emit_mask_finish():
            nc.vector.scalar_tensor_tensor(
                out=mask[:].rearrange("p a b -> p (a b)"),
                in0=dil[:].rearrange("p a b -> p (a b)"),
                scalar=-1.0,
                in1=umax[:].rearrange("p a b -> p (a b)"),
                op0=Alu.add, op1=Alu.add,
            )
            # zero mask outside the image: pad cols, partitions, junk rows
            nc.gpsimd.memset(mask[:, :, 0:RADIUS], 0.0)
            nc.gpsimd.memset(mask[:, :, RADIUS + W : PW], 0.0)
            nc.gpsimd.memset(mask[0:2, :, :], 0.0)
            nc.sync.dma_start(out=mask[119:120, 1:3, :], in_=zeros[:, 0 : 2 * PW])
            # mask sum per partition (placed mid-loop, fills a DVE gap)
            nc.vector.tensor_reduce(
                out=sums[:, 1:2], in_=mask[:].rearrange("p a b -> p (a b)"),
                axis=mybir.AxisListType.X, op=Alu.add,
            )

        # ---- identity + shifted identities for PE accumulation ----
        # ident_shift[a][k, m] = 1 iff m == k + a  (out[m] += P[m - a])
        rowidx = persist.tile([NP, NP], i16, tag="rowidx")
        pidx = persist.tile([NP, 1], mybir.dt.int32, tag="pidx")
        pidxf = persist.tile([NP, 1], f32, tag="pidxf")
        nc.gpsimd.iota(rowidx[:], pattern=[[1, NP]], base=0, channel_multiplier=0)
        nc.gpsimd.iota(pidx[:], pattern=[[1, 1]], base=0, channel_multiplier=1)
        nc.vector.tensor_copy(out=pidxf[:], in_=pidx[:])
        ident_shift = []
        for a in range(3):
            ida = persist.tile([NP, NP], f16, tag=f"ident{a}", name=f"ident{a}")
            pa = pidxf
            if a > 0:
                pa = persist.tile([NP, 1], f32, tag=f"pidxf{a}", name=f"pidxf{a}")
                nc.vector.tensor_scalar(
                    out=pa[:], in0=pidxf[:], scalar1=float(a), scalar2=None,
                    op0=Alu.add,
                )
            nc.vector.tensor_scalar(
                out=ida[:], in0=rowidx[:], scalar1=pa[:], scalar2=None,
                op0=Alu.is_equal,
            )
            ident_shift.append(ida)
        ident = ident_shift[0]

        pp = ctx.enter_context(tc.tile_pool(name="ps", bufs=1, space="PSUM"))
        psA = pp.tile([NP, 3, 512], f32, tag="psA")

        tmp = ctx.enter_context(tc.tile_pool(name="tmp", bufs=4))

        # half set: sy>0 all sx; sy=0 positive sx
        pairs = [(0, sx) for sx in range(1, 6)]
        pairs += [(sy, sx) for sy in range(1, 6) for sx in range(-5, 6)]

        n_mm = 0

        def acc(out, lhsT, rhs, last):
            nonlocal n_mm
            # first pair's direct j=0,1,2 reset the three psA banks
            nc.tensor.matmul(
                out=out, lhsT=lhsT, rhs=rhs,
                start=(n_mm < 3), stop=last,
                skip_group_check=True,
            )
            n_mm += 1

        import math

        SQRT_ALPHA = math.sqrt(200.0)

        for pi, (sy, sx) in enumerate(pairs):
            if pi == 28:
                emit_ladder(lbl, dil)
            elif pi == 46:
                emit_ladder(u, umax)
            elif pi == 58:
                emit_mask_finish()
            # Column trim: pair columns q (global col q-5) are only needed on
            # the union of the mask-alive direct range [5,357) and the mirror
            # source range [5-sx, 357-sx); compute [lo, lo+wid) with lo even
            # (keeps the window views parity-aligned). Pair 0 runs full width
            # so its start=True matmuls reset every psA column.
            if pi == 0:
                lo, wid = 0, PW
            elif sx > 0:
                lo = (5 - sx) & ~1
                wid = 357 - lo
            else:
                lo = 4
                wid = 353 - sx

            off_par = (5 + sx) % 2
            if off_par == 0:
                c0 = 5 + sx + lo
                winr = ch4[:, 0:3, 5 + sy : 8 + sy, c0 : c0 + wid]
            else:
                c0 = 4 + sx + lo
                winr = cho[:, 0:3, sy : 3 + sy, c0 : c0 + wid]
            ctr = cho[:, 0:3, 0:3, 4 + lo : 4 + lo + wid]

            d4 = tmp.tile([NP, 3, 3, PW], f16, tag="d4")
            nc.vector.tensor_sub(d4[:, :, :, 0:wid], winr, ctr)
            # per-channel Gaussian in one ACT pass:
            # DerErf(sqrt(200)*d) = (2/sqrt(pi)) * exp(-200*d^2)
            # (the constant (sqrt(pi)/2)^3 is folded in on the host)
            G = tmp.tile([NP, 3, 3, PW], f16, tag="G")
            nc.scalar.activation(out=G[:, :, :, 0:wid], in_=d4[:, :, :, 0:wid],
                                 func=Act.Derivative_Erf, scale=SQRT_ALPHA)
            Ga = tmp.tile([NP, 3, PW], f16, tag="Ga")
            eng_m1 = nc.gpsimd if pi % 2 == 0 else nc.vector
            eng_m1.tensor_mul(Ga[:, :, 0:wid], G[:, 0, :, 0:wid],
                              G[:, 1, :, 0:wid])
            wgt = tmp.tile([NP, 3, PW], f16, tag="wgt")
            nc.vector.tensor_mul(wgt[:, :, 0:wid], Ga[:, :, 0:wid],
                                 G[:, 2, :, 0:wid])

            dsal = tmp.tile([NP, 3, PW], f16, tag="dsal")
            nc.gpsimd.tensor_sub(
                dsal[:, :, 0:wid],
                ch4[:, 3, 5 + sy : 8 + sy, 5 + sx + lo : 5 + sx + lo + wid],
                ch4[:, 3, 5:8, 5 + lo : 5 + lo + wid],
            )
            adsal = tmp.tile([NP, 3, PW], f16, tag="adsal")
            if pi % 3 == 2:
                nc.vector.tensor_scalar(
                    out=adsal[:, :, 0:wid].bitcast(mybir.dt.uint16),
                    in0=dsal[:, :, 0:wid].bitcast(mybir.dt.uint16),
                    scalar1=0x7FFF, scalar2=None, op0=Alu.bitwise_and,
                )
            else:
                nc.scalar.activation(out=adsal[:, :, 0:wid],
                                     in_=dsal[:, :, 0:wid], func=Act.Abs)
            P = tmp.tile([NP, 3, PW], f16, tag="P")
            nc.vector.tensor_mul(P[:, :, 0:wid], wgt[:, :, 0:wid],
                                 adsal[:, :, 0:wid])

            # PE accumulation (one matmul per psA bank; a matmul's PSUM
            # output may not span banks). Direct, then mirror: row-shift by
            # sy via shifted-identity lhsT (psA[p,j] += P[p+dp, jp],
            # jp=(j-sy)%3, dp=(j-sy-jp)/3); col-shift by -sx via rhs view.
            mms = [(0, psA[:, j, lo : lo + wid], ident[:], P[:, j, 0:wid])
                   for j in range(3)]
            mlo = RADIUS - sx - lo  # mirror source col within the trimmed P
            for j in range(3):
                jp = (j - sy) % 3
                dp = (j - sy - jp) // 3
                a = -dp
                mms.append((a, psA[:, j, RADIUS : RADIUS + W],
                            ident_shift[a][:],
                            P[:, jp, mlo : mlo + W]))
            mms.sort(key=lambda t: t[0])
            is_last_pair = (sy, sx) == pairs[-1]
            for mi, (_, out_ap, lhsT, rhs) in enumerate(mms):
                acc(out_ap, lhsT, rhs, is_last_pair and mi == len(mms) - 1)

        # ---- masked partial sum: evacuate PSUM with the mask multiply
        # fused into the same tensor_tensor, then reduce ----
        scratch = persist.tile([NP, 3, PW], f16, tag="scratch")
        nc.vector.tensor_mul(scratch[:], psA[:, 0:3, 0:PW], mask[:])
        nc.vector.tensor_reduce(
            out=sums[:, 0:1], in_=scratch[:].rearrange("p a b -> p (a b)"),
            axis=mybir.AxisListType.X, op=Alu.add,
        )
        nc.sync.dma_start(out=out_d.ap(), in_=sums[:])

    nc.compile()
    return nc


def kernel(pred, feat):
    import os

    # A stale PJRT compilation-cache hit was observed to return a bad
    # executable (NaN result); force a fresh compile per process.
    os.environ.setdefault("JAX_ENABLE_COMPILATION_CACHE", "false")
    try:
        import jax

        jax.config.update("jax_enable_compilation_cache", False)
    except Exception:
        pass

    if "nc" not in _CACHE:
        _CACHE["nc"] = _build_kernel()
    nc = _CACHE["nc"]
    from concourse.bass_utils import run_bass_kernel_spmd

    pred = np.asarray(pred, dtype=np.float32).reshape(N_CORES, H, W)
    feat = np.asarray(feat, dtype=np.float32).reshape(N_CORES, 3, H, W)
    predp = np.zeros((N_CORES, 370, PADW2), np.float16)
    predp[:, 11:363, 10:362] = pred.astype(np.float16)
    featp = np.zeros((N_CORES, 3, 370, PADW2), np.float16)
    featp[:, :, 11:363, 10:362] = feat.astype(np.float16)
    in_maps = [
        {"pred16": np.ascontiguousarray(predp[i]),
         "feat16": np.ascontiguousarray(featp[i])}
        for i in range(N_CORES)
    ]
    res = run_bass_kernel_spmd(nc, in_maps, core_ids=list(range(N_CORES)))
    _CACHE["last_results"] = res
    tot = np.zeros(2, np.float64)
    for r in res.results:
        tot += r["partial"].astype(np.float64).sum(axis=0)
    # kernel accumulates (2/sqrt(pi))^3 * exp(-alpha*ssq) via Derivative_Erf
    import math

    tot[0] *= (math.sqrt(math.pi) / 2.0) ** 3
    loss = tot[0] / (tot[1] + 1e-6)
    return np.array(loss, dtype=np.float32)
